# revision 1
# baseline (speedup 1.0000x reference)
"""Trainium2 Bass kernel for BroadcastResidualBlock.

Reference computation (per image, NHWC, H=W=19, C=256, HW=361):
    h1 = relu(bn1(x @ conv1_w + conv1_b))          # 1x1 conv = channel mix
    h2 = relu(dense(h1 over flattened board))       # spatial mix, per channel
    h3 = relu(bn2(h2 @ conv2_w + conv2_b))          # 1x1 conv
    out = x + h3

Strategy: pure data parallel over batch N=256 -> 32 images per core on 8
cores.  BN (inference) folds into the conv weights/biases on the host.  The
host also pre-transposes x into "C-layout" (N, C, HW) so every device-side
matmul contracts over the partition dimension with zero on-device transposes:

    s1: psum[r,  d] += xC_bf16[c_chunk, r_chunk].T @ w1[c_chunk, d]   (h1: S-layout)
    s2: psum[c,  q] += h1[p_chunk, c_chunk].T     @ dw[p_chunk, q]    (h2: C-layout)
    s3: psum[d,  q] += w2[c_chunk, d_chunk].T     @ h2[c_chunk, q]    (h3: C-layout)
    out = relu(psum3) + xC   (fused on VectorE), stored in C-layout.

Matmuls run in bf16 (fp32 PSUM accumulation); x stays fp32 for the residual.
The host transposes the output back to NHWC.
"""

import numpy as np
import ml_dtypes

import concourse.bass as bass
import concourse.mybir as mybir
import concourse.tile as tile
from concourse import bacc
from concourse.bass_utils import run_bass_kernel_spmd

N_CORES = 8
NIMG = 32            # images per core
C = 256
HW = 361             # 19*19
P = 128
EPS = 1e-3

F32 = mybir.dt.float32
BF16 = mybir.dt.bfloat16
AF = mybir.ActivationFunctionType
ALU = mybir.AluOpType

_prog_cache = {}


def build_program(has_b1: bool, has_b2: bool, has_b3: bool, reps: int = 1):
    nc = bacc.Bacc("TRN2", target_bir_lowering=False, debug=False)

    xc = nc.dram_tensor("xc", [NIMG, 2, P, HW], F32, kind="ExternalInput").ap()
    w1 = nc.dram_tensor("w1", [2, P, C], BF16, kind="ExternalInput").ap()
    dw = nc.dram_tensor("dw", [3, P, HW], BF16, kind="ExternalInput").ap()
    w2 = nc.dram_tensor("w2", [2, P, C], BF16, kind="ExternalInput").ap()
    b1 = b2 = b3 = None
    if has_b1:
        b1 = nc.dram_tensor("b1", [P, C], F32, kind="ExternalInput").ap()
    if has_b2:
        b2 = nc.dram_tensor("b2", [P, HW], F32, kind="ExternalInput").ap()
    if has_b3:
        b3 = nc.dram_tensor("b3", [2, P], F32, kind="ExternalInput").ap()
    yc = nc.dram_tensor("yc", [NIMG, 2, P, HW], F32, kind="ExternalOutput").ap()

    with tile.TileContext(nc) as tc:
        with (
            tc.tile_pool(name="const", bufs=1) as cpool,
            tc.tile_pool(name="xf", bufs=5) as xf_pool,
            tc.tile_pool(name="xb", bufs=3) as xb_pool,
            tc.tile_pool(name="h1", bufs=3) as h1_pool,
            tc.tile_pool(name="h2", bufs=3) as h2_pool,
            tc.tile_pool(name="yo", bufs=3) as yo_pool,
            tc.tile_pool(name="ps", bufs=8, space="PSUM") as ps_pool,
        ):
            w1sb = cpool.tile([P, 2, C], BF16)
            nc.sync.dma_start(w1sb[:], w1.rearrange("co ci d -> ci co d"))
            dwsb = cpool.tile([P, 3, HW], BF16)
            nc.sync.dma_start(dwsb[:], dw.rearrange("po pi q -> pi po q"))
            w2sb = cpool.tile([P, 2, C], BF16)
            nc.sync.dma_start(w2sb[:], w2.rearrange("co ci d -> ci co d"))
            b1sb = b2sb = b3sb = None
            if has_b1:
                b1sb = cpool.tile([P, C], F32)
                nc.sync.dma_start(b1sb[:], b1)
            if has_b2:
                b2sb = cpool.tile([P, HW], F32)
                nc.sync.dma_start(b2sb[:], b2)
            if has_b3:
                b3sb = cpool.tile([P, 2], F32)
                nc.sync.dma_start(b3sb[:], b3.rearrange("co ci -> ci co"))

            def emit_s1(i):
                xf = xf_pool.tile([P, 2, HW], F32, tag="xf")
                nc.sync.dma_start(xf[:], xc[i].rearrange("co ci q -> ci co q"))
                xb = xb_pool.tile([P, 2, HW], BF16, tag="xb")
                nc.vector.tensor_copy(xb[:], xf[:])
                h1 = h1_pool.tile([P, 3, C], BF16, tag="h1")
                for rc in range(3):
                    m = 128 if rc < 2 else 105
                    ps = ps_pool.tile([P, 512], F32, tag="ps")
                    for cc in range(2):
                        nc.tensor.matmul(
                            ps[:m, :C],
                            xb[:, cc, rc * 128 : rc * 128 + m],
                            w1sb[:, cc, :],
                            start=(cc == 0),
                            stop=(cc == 1),
                        )
                    if b1sb is not None:
                        nc.vector.scalar_tensor_tensor(
                            ps[:m, :C], ps[:m, :C], 0.0, b1sb[:m, :],
                            ALU.bypass, ALU.add,
                        )
                    nc.scalar.activation(h1[:m, rc, :], ps[:m, :C], AF.Relu)
                return xf, h1

            def emit_s2(i, h1):
                h2 = h2_pool.tile([P, 2, HW], BF16, tag="h2")
                for cc in range(2):
                    ps = ps_pool.tile([P, 512], F32, tag="ps")
                    for pc in range(3):
                        k = 128 if pc < 2 else 105
                        nc.tensor.matmul(
                            ps[:, :HW],
                            h1[:k, pc, cc * 128 : (cc + 1) * 128],
                            dwsb[:k, pc, :],
                            start=(pc == 0),
                            stop=(pc == 2),
                        )
                    if b2sb is not None:
                        nc.vector.scalar_tensor_tensor(
                            ps[:, :HW], ps[:, :HW], 0.0, b2sb[:],
                            ALU.bypass, ALU.add,
                        )
                    nc.scalar.activation(h2[:, cc, :], ps[:, :HW], AF.Relu)
                return h2

            def emit_s3(i, xf, h2):
                yo = yo_pool.tile([P, 2, HW], F32, tag="yo")
                for dc in range(2):
                    ps = ps_pool.tile([P, 512], F32, tag="ps")
                    for cc in range(2):
                        nc.tensor.matmul(
                            ps[:, :HW],
                            w2sb[:, cc, dc * 128 : (dc + 1) * 128],
                            h2[:, cc, :],
                            start=(cc == 0),
                            stop=(cc == 1),
                        )
                    if b3sb is not None:
                        nc.scalar.activation(
                            yo[:, dc, :], ps[:, :HW], AF.Relu,
                            bias=b3sb[:, dc : dc + 1],
                        )
                        nc.vector.tensor_add(yo[:, dc, :], yo[:, dc, :], xf[:, dc, :])
                    else:
                        nc.vector.scalar_tensor_tensor(
                            yo[:, dc, :], ps[:, :HW], 0.0, xf[:, dc, :],
                            ALU.max, ALU.add,
                        )
                nc.sync.dma_start(yc[i].rearrange("co ci q -> ci co q"), yo[:])

            def body():
                # software pipeline: s1(i) | s2(i-1) | s3(i-2) so every PE
                # input was produced a full pipeline step earlier
                xfs, h1s, h2s = {}, {}, {}
                for step in range(NIMG + 2):
                    if step >= 2:
                        emit_s3(step - 2, xfs.pop(step - 2), h2s.pop(step - 2))
                    if 1 <= step <= NIMG:
                        h2s[step - 1] = emit_s2(step - 1, h1s.pop(step - 1))
                    if step < NIMG:
                        xfs[step], h1s[step] = emit_s1(step)

            if reps == 1:
                body()
            else:
                with tc.For_i(0, reps, 1):
                    body()

    nc.compile()
    return nc


def _get_program(key):
    if key not in _prog_cache:
        _prog_cache[key] = build_program(*key)
    return _prog_cache[key]


def _marshal(x, conv1_w, conv1_b, bn1_mean, bn1_var, bn1_beta,
             dense_w, dense_b, conv2_w, conv2_b, bn2_mean, bn2_var, bn2_beta):
    bf16 = ml_dtypes.bfloat16
    n = x.shape[0]
    rs1 = 1.0 / np.sqrt(bn1_var.astype(np.float64) + EPS)
    rs2 = 1.0 / np.sqrt(bn2_var.astype(np.float64) + EPS)
    w1f = conv1_w.astype(np.float64) * rs1[None, :]
    w2f = conv2_w.astype(np.float64) * rs2[None, :]
    b1f = (conv1_b - bn1_mean).astype(np.float64) * rs1 + bn1_beta
    b2f = dense_b.astype(np.float64)
    b3f = (conv2_b - bn2_mean).astype(np.float64) * rs2 + bn2_beta
    has_b1 = bool(np.any(b1f != 0.0))
    has_b2 = bool(np.any(b2f != 0.0))
    has_b3 = bool(np.any(b3f != 0.0))

    w1b = np.ascontiguousarray(w1f.astype(bf16).reshape(2, P, C))
    dwp = np.zeros((3 * P, HW), np.float64)
    dwp[:HW] = dense_w
    dwb = np.ascontiguousarray(dwp.astype(bf16).reshape(3, P, HW))
    w2b = np.ascontiguousarray(w2f.astype(bf16).reshape(2, P, C))

    x_c = np.ascontiguousarray(
        x.reshape(n, HW, C).transpose(0, 2, 1)
    ).reshape(N_CORES, NIMG, 2, P, HW)

    in_maps = []
    for c in range(N_CORES):
        m = {"xc": x_c[c], "w1": w1b, "dw": dwb, "w2": w2b}
        if has_b1:
            m["b1"] = np.ascontiguousarray(
                np.broadcast_to(b1f.astype(np.float32), (P, C)))
        if has_b2:
            m["b2"] = np.ascontiguousarray(
                np.broadcast_to(b2f.astype(np.float32), (P, HW)))
        if has_b3:
            m["b3"] = np.ascontiguousarray(
                b3f.astype(np.float32).reshape(2, P))
        in_maps.append(m)
    return (has_b1, has_b2, has_b3), in_maps


def _unmarshal(results, n, h, w):
    y = np.stack([results[c]["yc"] for c in range(N_CORES)])
    y = y.reshape(n, C, HW).transpose(0, 2, 1)
    return np.ascontiguousarray(y.reshape(n, h, w, C).astype(np.float32))


def kernel(x, conv1_w, conv1_b, bn1_mean, bn1_var, bn1_beta,
           dense_w, dense_b, conv2_w, conv2_b, bn2_mean, bn2_var, bn2_beta):
    n, h, w, _ = x.shape
    flags, in_maps = _marshal(
        x, conv1_w, conv1_b, bn1_mean, bn1_var, bn1_beta,
        dense_w, dense_b, conv2_w, conv2_b, bn2_mean, bn2_var, bn2_beta)
    nc = _get_program((*flags, 1))
    res = run_bass_kernel_spmd(nc, in_maps, list(range(N_CORES)))
    return _unmarshal(res.results, n, h, w)
